# revision 2
# baseline (speedup 1.0000x reference)
"""Trainium2 Bass kernel for the ButterflyMlp problem.

Computes log_softmax(L3(relu(L2(relu(L1(x)))))) where each Li is a masked
linear layer (butterfly sparsity: global column stripes + a diagonal band),
batch 65536, data-parallel over 8 NeuronCores (8192 rows/core).

Strategy (per core, feature-major):
  - Masks are pre-applied to weights on host. Layer-1 exploits the butterfly
    structure: the stripe columns (mask true for every output row) form a
    dense [|S|, 784] GEMM shared by all outputs, and the per-output-block
    band adds one narrow [|R_j|<=128, 112] GEMM per 112-row output block.
    This cuts layer-1 matmul passes from 49 to 21 per batch chunk.
  - All GEMMs run in float32r (1 cycle/row on the PE for N>=256, ~1.6e-4
    relative error vs 4 cycles/row for plain fp32).
  - Activations stay on-chip feature-major [feat, batch]; ReLU+bias is fused
    into the PSUM->SBUF eviction, alternating ScalarE/VectorE.
  - The [10, batch] layer-3 output is transposed to batch-major via PE
    matmuls against a 10x10 identity, then log_softmax (no max subtraction
    needed: logits are O(1)) runs on [128, 640] tiles.
  - Output returned as [128, 64, 10] per core; host reassembles [B, 10].
"""
import sys
sys.path.insert(0, "/opt/trn_rl_repo")
import numpy as np

import concourse.bass as bass
import concourse.bacc as bacc
import concourse.mybir as mybir
import concourse.tile as tile
from concourse import bass_utils

F32 = mybir.dt.float32
F32R = mybir.dt.float32r
AF = mybir.ActivationFunctionType
ALU = mybir.AluOpType

N_CORES = 8
NB = 512          # batch columns per matmul (one PSUM bank of fp32)
OT = 112          # layer-1 output block width (784/7; band window fits 128)


def _decompose_mask1(mask1):
    """Split the butterfly mask into stripe columns S (true for every row)
    and per-output-block residual columns R_j."""
    D_out, D_in = mask1.shape
    S = np.where(mask1.all(axis=0))[0]
    n_blk = (D_out + OT - 1) // OT
    stripe_set = np.zeros(D_in, dtype=bool)
    stripe_set[S] = True
    R_list = []
    for j in range(n_blk):
        blk = mask1[j * OT:(j + 1) * OT]
        cols = np.where(blk.any(axis=0) & ~stripe_set)[0]
        assert len(cols) <= 128, f"band block {j} has {len(cols)} cols"
        R_list.append(cols)
    return S, R_list


def _build_program(meta):
    """Build the Bass program. meta: dict with sizes."""
    nS, nR_tot, R_lens = meta["nS"], meta["nR_tot"], meta["R_lens"]
    Bc = meta["Bc"]
    D1, H, C = meta["D1"], meta["H"], meta["C"]
    n_blk = len(R_lens)
    n_sc = (nS + 127) // 128              # stripe K-chunks
    sc_sizes = [nS // n_sc + (1 if i < nS % n_sc else 0) for i in range(n_sc)]
    sc_off = np.cumsum([0] + sc_sizes)
    nch = Bc // NB                        # batch chunks per core
    n_kc2 = D1 // OT                      # layer-2 K chunks (= n_blk)
    GRP = Bc // 128                       # groups of 10 in the [128, *] output

    nc = bacc.Bacc("TRN2", target_bir_lowering=False, debug=False,
                   enable_asserts=False, num_devices=N_CORES)

    xs_d = nc.dram_tensor("xs", [nS, Bc], F32R, kind="ExternalInput").ap()
    xb_d = nc.dram_tensor("xb", [nR_tot, Bc], F32R, kind="ExternalInput").ap()
    ws_d = nc.dram_tensor("ws", [nS, D1], F32R, kind="ExternalInput").ap()
    wb_d = nc.dram_tensor("wb", [128, n_blk * OT], F32R, kind="ExternalInput").ap()
    w2_d = nc.dram_tensor("w2", [OT, n_kc2 * H], F32R, kind="ExternalInput").ap()
    w3_d = nc.dram_tensor("w3", [H, C], F32R, kind="ExternalInput").ap()
    b1_d = nc.dram_tensor("b1", [OT, n_blk], F32, kind="ExternalInput").ap()
    b2_d = nc.dram_tensor("b2", [H, 1], F32, kind="ExternalInput").ap()
    b3_d = nc.dram_tensor("b3", [C, 1], F32, kind="ExternalInput").ap()
    id_d = nc.dram_tensor("ident", [C, C], F32, kind="ExternalInput").ap()
    out_d = nc.dram_tensor("out", [128, GRP * C], F32, kind="ExternalOutput").ap()

    R_off = np.cumsum([0] + R_lens)

    with tile.TileContext(nc) as tc:
        with tc.tile_pool(name="wp", bufs=1) as wp, \
             tc.tile_pool(name="xp", bufs=3) as xp, \
             tc.tile_pool(name="hp", bufs=2) as hp, \
             tc.tile_pool(name="op", bufs=1) as op, \
             tc.tile_pool(name="ps1", bufs=3, space="PSUM") as ps1, \
             tc.tile_pool(name="ps2", bufs=2, space="PSUM") as ps2, \
             tc.tile_pool(name="ps3", bufs=1, space="PSUM") as ps3, \
             tc.tile_pool(name="pst", bufs=2, space="PSUM") as pst:

            # ---- resident weights ----
            ws_sb = []
            for c in range(n_sc):
                t = wp.tile([sc_sizes[c], D1], F32R, name=f"ws_sb{c}")
                nc.sync.dma_start(t[:], ws_d[sc_off[c]:sc_off[c + 1], :])
                ws_sb.append(t)
            wb_sb = wp.tile([128, n_blk * OT], F32R)
            nc.sync.dma_start(wb_sb[:], wb_d[:])
            w2_sb = wp.tile([OT, n_kc2 * H], F32R)
            nc.sync.dma_start(w2_sb[:], w2_d[:])
            w3_sb = wp.tile([H, C], F32R)
            nc.sync.dma_start(w3_sb[:], w3_d[:])
            b1_sb = wp.tile([OT, n_blk], F32)
            nc.sync.dma_start(b1_sb[:], b1_d[:])
            b2_sb = wp.tile([H, 1], F32)
            nc.sync.dma_start(b2_sb[:], b2_d[:])
            b3_sb = wp.tile([C, 1], F32)
            nc.sync.dma_start(b3_sb[:], b3_d[:])
            id_sb = wp.tile([C, C], F32)
            nc.sync.dma_start(id_sb[:], id_d[:])

            y3bm = op.tile([128, GRP * C], F32)

            for n in range(nch):
                bs = n * NB
                # ---- x loads (feature-major, pre-gathered rows) ----
                xs_t = []
                for c in range(n_sc):
                    t = xp.tile([sc_sizes[c], NB], F32R, name=f"xs_t{c}",
                                tag=f"xs{c}")
                    nc.sync.dma_start(t[:], xs_d[sc_off[c]:sc_off[c + 1],
                                                 bs:bs + NB])
                    xs_t.append(t)
                xb_t = []
                for j in range(n_blk):
                    t = xp.tile([R_lens[j], NB], F32R, name=f"xb_t{j}",
                                tag=f"xb{j}")
                    nc.sync.dma_start(t[:], xb_d[R_off[j]:R_off[j + 1],
                                                 bs:bs + NB])
                    xb_t.append(t)

                # ---- layer 1: 7 output blocks, K = stripes + band ----
                y1_t = []
                for j in range(n_blk):
                    p = ps1.tile([OT, NB], F32, tag="l1")
                    for c in range(n_sc):
                        nc.tensor.matmul(p[:], ws_sb[c][:, j * OT:(j + 1) * OT],
                                         xs_t[c][:], start=(c == 0), stop=False)
                    nc.tensor.matmul(p[:], wb_sb[:R_lens[j],
                                                 j * OT:(j + 1) * OT],
                                     xb_t[j][:], start=False, stop=True)
                    h = hp.tile([OT, NB], F32R, name=f"y1_{j}", tag=f"y1{j}")
                    if j % 2 == 0:
                        nc.vector.tensor_scalar(h[:], p[:], b1_sb[:, j:j + 1],
                                                0.0, op0=ALU.add, op1=ALU.max)
                    else:
                        nc.scalar.activation(h[:], p[:], AF.Relu,
                                             bias=b1_sb[:, j:j + 1])
                    y1_t.append(h)

                # ---- layer 2: K = 784 over 7 chunks of 112 ----
                p2 = ps2.tile([H, NB], F32, tag="l2")
                for k in range(n_kc2):
                    nc.tensor.matmul(p2[:], w2_sb[:, k * H:(k + 1) * H],
                                     y1_t[k][:], start=(k == 0),
                                     stop=(k == n_kc2 - 1))
                y2 = hp.tile([H, NB], F32R, tag="y2")
                nc.scalar.activation(y2[:], p2[:], AF.Relu, bias=b2_sb[:, 0:1])

                # ---- layer 3: K = 128, out [10, NB] ----
                p3 = ps3.tile([C, NB], F32, tag="l3")
                nc.tensor.matmul(p3[:], w3_sb[:], y2[:], start=True, stop=True)
                y3t = hp.tile([C, NB], F32, tag="y3t")
                nc.scalar.activation(y3t[:], p3[:], AF.Identity,
                                     bias=b3_sb[:, 0:1])

                # ---- transpose [10, NB] -> 4x [128, 10] via PE ----
                ntp = NB // 128
                tp = pst.tile([128, ntp * C], F32, tag="tp")
                for c4 in range(ntp):
                    nc.tensor.matmul(tp[:, c4 * C:(c4 + 1) * C],
                                     y3t[:, c4 * 128:(c4 + 1) * 128],
                                     id_sb[:], start=True, stop=True)
                g0 = (bs // 128) * C
                nc.vector.tensor_copy(y3bm[:, g0:g0 + ntp * C], tp[:])

            # ---- log_softmax over class groups of 10 ----
            ex = op.tile([128, GRP * C], F32)
            nc.scalar.activation(ex[:], y3bm[:], AF.Exp)
            s = op.tile([128, GRP], F32)
            nc.vector.tensor_reduce(s[:], ex[:].rearrange("p (g c) -> p g c", c=C),
                                    axis=mybir.AxisListType.X, op=ALU.add)
            ls = op.tile([128, GRP], F32)
            nc.scalar.activation(ls[:], s[:], AF.Ln)
            o = op.tile([128, GRP * C], F32)
            nc.vector.tensor_tensor(o[:].rearrange("p (g c) -> p g c", c=C),
                                    y3bm[:].rearrange("p (g c) -> p g c", c=C),
                                    ls[:].to_broadcast([128, GRP, C]),
                                    op=ALU.subtract)
            nc.sync.dma_start(out_d[:], o[:])

    nc.compile()
    return nc


_CACHE = {}


def _prepare(x, W1, b1, W2, b2, W3, b3, mask1, mask2, mask3):
    B, D1 = x.shape
    H = W2.shape[0]
    C = W3.shape[0]
    assert B % N_CORES == 0
    Bc = B // N_CORES

    S, R_list = _decompose_mask1(np.asarray(mask1))
    R_lens = [len(r) for r in R_list]
    n_blk = len(R_list)

    Wm1 = (np.asarray(W1) * np.asarray(mask1)).astype(np.float32)
    Wm2 = (np.asarray(W2) * np.asarray(mask2)).astype(np.float32)
    Wm3 = (np.asarray(W3) * np.asarray(mask3)).astype(np.float32)

    ws = np.ascontiguousarray(Wm1[:, S].T)                # [|S|, D1]
    wb = np.zeros((128, n_blk * OT), np.float32)
    for j, R in enumerate(R_list):
        wb[:len(R), j * OT:j * OT + OT] = Wm1[j * OT:(j + 1) * OT, R].T
    n_kc2 = D1 // OT
    w2 = np.ascontiguousarray(
        Wm2.T.reshape(n_kc2, OT, H).transpose(1, 0, 2).reshape(OT, n_kc2 * H))
    w3 = np.ascontiguousarray(Wm3.T)                      # [H, C]
    b1p = np.ascontiguousarray(
        np.asarray(b1, np.float32).reshape(n_blk, OT).T)  # [OT, n_blk]
    b2p = np.asarray(b2, np.float32).reshape(H, 1)
    b3p = np.asarray(b3, np.float32).reshape(C, 1)
    ident = np.eye(C, dtype=np.float32)

    xT = np.asarray(x, np.float32).T                      # [D1, B] view
    xs_all = np.ascontiguousarray(xT[S])                  # [|S|, B]
    R_cat = np.concatenate(R_list)
    xb_all = np.ascontiguousarray(xT[R_cat])              # [sum R, B]

    meta = dict(nS=len(S), nR_tot=len(R_cat), R_lens=R_lens,
                Bc=Bc, D1=D1, H=H, C=C)
    key = (B, D1, H, C, len(S), tuple(R_lens))
    if key not in _CACHE:
        _CACHE[key] = _build_program(meta)
    nc = _CACHE[key]

    in_maps = []
    for c in range(N_CORES):
        sl = slice(c * Bc, (c + 1) * Bc)
        in_maps.append({
            "xs": np.ascontiguousarray(xs_all[:, sl]),
            "xb": np.ascontiguousarray(xb_all[:, sl]),
            "ws": ws, "wb": wb, "w2": w2, "w3": w3,
            "b1": b1p, "b2": b2p, "b3": b3p, "ident": ident,
        })
    return nc, in_maps, meta


def _assemble(results, meta):
    B = meta["Bc"] * N_CORES
    C = meta["C"]
    outs = []
    for c in range(N_CORES):
        o = results[c]["out"]                             # [128, GRP*C]
        outs.append(o.reshape(128, -1, C).transpose(1, 0, 2).reshape(-1, C))
    return np.concatenate(outs, axis=0).astype(np.float32)


def kernel(**inputs):
    nc, in_maps, meta = _prepare(**inputs)
    res = bass_utils.run_bass_kernel_spmd(nc, in_maps,
                                          core_ids=list(range(N_CORES)))
    return _assemble(res.results, meta)


def kernel_traced(tmpdir=None, **inputs):
    """Same as kernel() but with NTFF profiling; returns (output, results)."""
    nc, in_maps, meta = _prepare(**inputs)
    res = bass_utils.run_bass_kernel_spmd(nc, in_maps,
                                          core_ids=list(range(N_CORES)),
                                          trace=True, tmpdir=tmpdir)
    return _assemble(res.results, meta), res
